# revision 21
# baseline (speedup 1.0000x reference)
"""Trainium2 Bass kernel for the CP-PINN tensor reconstruction problem.

Computes, for xs (3,320,1) and three per-axis MLP weight stacks:
    f_d = MLP_d(xs[d])            (320, 64)   [tanh MLP: 1->128->128->128->64]
    out[a,b,c] = sum_r f_0[a,r] * f_1[b,r] * f_2[c,r]   ->  (320, 320, 320) f32

Strategy: data-parallel over the output's first axis across 8 NeuronCores
(40 a-points per core, no collectives). The output stream is fp16 (the
measured rel-L2 error vs the f32 reference is ~6e-4, far under the 2e-2
gate), halving the HBM write floor from ~45.8us to ~22.9us per core.
Each core:
  - loads weights+biases+x with ONE host-packed DMA (x is packed into
    row 0 of the weight tensor), then makes a single f32r-rounded copy
    wp_r: fp32 matmuls run at 1/4 PE rate, float32r at full rate for
    free-dim >= 256, but every f32r-matmul input must be *produced*
    rounded (and f32r matmuls cannot write PSUM at a partition offset,
    so w3 is packed [w3 | w3] to fill both halves in one matmul);
  - computes the three MLPs (f32r matmuls on PE, tanh on ScalarE,
    interleaved layer-by-layer; last hidden + final layer fused per-dim),
    final-layer bias-adds on VectorE writing fp16 factor tiles
    duplicated across partition halves; f0 is packed as f0p (128, 20):
    rows 0-63 = f0[:, a], rows 64-127 = f0[:, a+20];
  - Khatri-Rao kr[r, a*N+b] = f0[r,a]*f1[r,b] in fp16 via 20 dual-half
    VectorE tensor_scalar_mul ops serving both output halves at once
    (GPSIMD was tried and is unusable: ~6us/op dispatch overhead);
  - reconstructs its (40*320, 320) fp16 slab in 25 "quads" = one
    256-row low chunk + one 256-row high chunk. Each partition owns TWO
    consecutive output rows (even/odd matmuls from stride-2 kr column
    views) so every DMA descriptor covers 1280 B of DRAM -- 640 B rows
    measured only ~180 GB/s vs ~330 GB/s at 1280 B. Per chunk: one
    2-bank PSUM pair-tile (4 in flight), 2 fp16 matmuls, one 2-block
    PSUM->fp16-SBUF copy (VectorE ~716ns / ScalarE ~1123ns, 29/21
    rate-balanced assignment; these copies are the only PSUM
    evacuation path and bound the stream together with the DMA),
    staged contiguously per stream; tapered DMA groups go out on BOTH
    HWDGE rings (one ring caps at ~220 GB/s): low rows from the
    otherwise-idle SP sequencer, high rows from ScalarE but emitted a
    few quads late so ACT's in-order queue never stalls on the wait.
"""

import sys

if "/opt/trn_rl_repo" not in sys.path:
    sys.path.insert(0, "/opt/trn_rl_repo")

import numpy as np

import concourse.bacc as bacc
import concourse.mybir as mybir
from concourse import tile
from concourse.bass_utils import run_bass_kernel_spmd

DIMS = 3
N = 320          # points per coordinate axis
R = 64           # CP rank
H = 128          # hidden width
NCORES = 8
NA = N // NCORES          # a-points per core (40)
NROWS = NA * N            # output rows per core (12800)
MCH = 128                 # (a,b)-rows per matmul chunk
NCHUNK = NROWS // MCH     # 100
NPAIR = NCHUNK // 2       # 50 low/high chunk pairs
NQUAD = NPAIR // 2        # 25 two-pair quads
GROUPS_Q = (1, 3, 6, 6, 5, 3, 1)   # quads per output DMA group
assert sum(GROUPS_Q) == NQUAD
GMAX = max(GROUPS_Q)

# Copy-engine assignment per PSUM pair-tile, sequence [lo_q0, hi_q0,
# lo_q1, hi_q1, ...]: 'v' = VectorE (~716ns/copy), 's' = ScalarE
# (~1123ns/copy). DVE also carries the KR stream + final adds; ACT the
# MLP head + hi-stream DMA issues. (A 4-bank/quad variant with single
# 4-block ACT copies measured WORSE: PSUM drops to 2 slots and the
# matmul->copy->matmul chain serializes.)
_NV = 29
COPY_ENG = tuple(
    'v' if i in {round(j * 50 / _NV) for j in range(_NV)} else 's'
    for i in range(50))

KR_ENGINE = "vector"   # "gpsimd" | "vector" (gpsimd: ~6us/op Q7 dispatch - unusable)

# Packed-weights column layout (one (128, WCOLS) f32 tensor):
#   [0,384)    w1 (3 x 128 cols)        [384,768)  w2
#   [768,1152) w3 duplicated: per dim 128 cols = [w3 | w3] so ONE f32r
#              matmul writes the factor into both partition halves
#              (f32r matmuls cannot target a PSUM partition offset)
#   [1152,1155) b0 [1155,1158) b1 [1158,1161) b2 [1161,1164) b3 (dup halves)
#   [1164,1548) w0 (row 0 only, 3 x 128 cols)   [1548,2228) packed x (row 0)
W1_OFF, W2_OFF, W3_OFF = 0, 384, 768
B0_OFF, B1_OFF, B2_OFF, B3_OFF = 1152, 1155, 1158, 1161
W0_OFF, WCOLS = 1164, 2228
XP_OFF = 1548
# Packed-x layout (row 0 of wp, from XP_OFF): x0(40) | x1(320) | x2(320)
X0_OFF, X1_OFF, X2_OFF, XCOLS = 0, NA, NA + N, NA + 2 * N

F32 = mybir.dt.float32
F32R = mybir.dt.float32r
F16 = mybir.dt.float16
TANH = mybir.ActivationFunctionType.Tanh

_PROG = None


def _build_program(loop=1, variant="full"):
    """loop>1 wraps the whole compute body in a Tile hardware For_i that
    repeats it `loop` times inside one NEFF launch — benchmarking only."""
    nc = bacc.Bacc("TRN2", target_bir_lowering=False)

    wp = nc.dram_tensor("wp", [H, WCOLS], F32, kind="ExternalInput")
    out = nc.dram_tensor("out", [NROWS, N], F16, kind="ExternalOutput")

    with tile.TileContext(nc) as tc:
        with (
            tc.tile_pool(name="consts", bufs=1) as consts,
            tc.tile_pool(name="work", bufs=2) as work,
            tc.tile_pool(name="stage", bufs=3) as stagep,
            tc.tile_pool(name="ps", bufs=4, space="PSUM") as psp,
        ):
            wp_sb = consts.tile([H, WCOLS], F32)
            nc.sync.dma_start(wp_sb[:], wp[:, :])
            # f32r-rounded copy: everything a matmul consumes (weights and
            # the packed x row) must be *produced* as f32r.
            wp_r = consts.tile([H, WCOLS], F32R)
            nc.vector.tensor_copy(wp_r[:], wp_sb[:])

            import contextlib
            loop_cm = (tc.For_i(0, loop, 1,
                                hint_engines=(mybir.EngineType.PE,),
                                staggered_reset=True)
                       if loop > 1 else contextlib.nullcontext())
            with loop_cm:
                _emit_body(nc, tc, consts, work, stagep, psp,
                           out, wp_sb, wp_r, variant)

    nc.compile()
    return nc


def _pair_copy_views(ps, stg, s, i):
    """(src, dst) for evacuating pair-tile ps (2 banks [even | odd], 320
    cols each at offsets 0/512) into stream region s (0=lo, 1=hi) slot i
    of the group staging tile — each DMA stream reads contiguously."""
    src = ps[:, :].rearrange("p (b x) -> p b x", x=512)[:, :, 0:N]
    dst = (stg[:, :].rearrange("p (s r) -> p s r", s=2)
           [:, s, i * 2 * N:(i + 1) * 2 * N]
           .rearrange("p (b c) -> p b c", c=N))
    return src, dst


def _emit_body(nc, tc, consts, work, stagep, psp, out, wp_sb, wp_r,
               variant="full"):
    # Each partition owns TWO consecutive output rows (j=0,1) so every
    # DMA descriptor covers 1280B of DRAM (640B rows halved throughput).
    outv = out[:, :].rearrange("(m p j) c -> p m (j c)", p=MCH, j=2)

    warm = work.tile([1, 1], F32, name="warm", tag="warm")
    nc.vector.memset(warm[:], 0.0)
    nc.scalar.activation(warm[:], warm[:], TANH)

    if variant == "empty":
        return

    if variant in ("dma_only", "dma_2ring", "cp_dve", "cp_act"):
        if variant in ("cp_dve", "cp_act"):
            ps0 = psp.tile([MCH, 1024], F32, name="ps0", tag="cps")
            for j in range(2):
                nc.scalar.copy(ps0[:, j * 512:(j + 1) * 512], wp_sb[:, 0:512])
        q = 0
        for gsz in GROUPS_Q:
            stg = stagep.tile([MCH, 2 * GMAX * 2 * N], F16, name="stg",
                              tag="stg")
            if variant in ("dma_only", "dma_2ring"):
                nc.vector.memset(stg[:, 0:1], 1.0)
            else:
                eng = (nc.vector.tensor_copy if variant == "cp_dve"
                       else nc.scalar.copy)
                for i in range(gsz):
                    for s in (0, 1):
                        src, dst = _pair_copy_views(ps0, stg, s, i)
                        eng(dst, src)
                q += gsz
                continue
            sv = stg[:, :].rearrange("p (s r) -> p s r", s=2)
            lo = sv[:, 0, 0:gsz * 2 * N].rearrange("p (m w) -> p m w", w=2 * N)
            hi = sv[:, 1, 0:gsz * 2 * N].rearrange("p (m w) -> p m w", w=2 * N)
            nc.sync.dma_start(outv[:, q:q + gsz, :], lo)
            (nc.scalar if variant == "dma_2ring" else nc.sync).dma_start(
                outv[:, NQUAD + q:NQUAD + q + gsz, :], hi)
            q += gsz
        return

    # f32 factor tiles, duplicated across both partition halves.
    # f0p: rows 0-63 = f0[:, j], rows 64-127 = f0[:, j+20].
    f0p = consts.tile([2 * R, NA // 2], F32)
    f1_sb = consts.tile([2 * R, N], F16)
    f2_sb = consts.tile([2 * R, N], F16)

    # The three MLPs interleaved layer-by-layer so PE never waits on the
    # ScalarEngine tanh of the same dim (PE executes in program order).
    dims = [(0, X0_OFF, NA), (1, X1_OFF, N), (2, X2_OFF, N)]
    h_cur = {d: wp_r[0:1, XP_OFF + xoff:XP_OFF + xoff + npts]
             for d, xoff, npts in dims}
    w_l0 = wp_r[0:1, :]
    for li, (w_off, b_off, w_ap, wid) in enumerate((
            (W0_OFF, B0_OFF, w_l0, H), (W1_OFF, B1_OFF, wp_r, H))):
        for d, _, npts in dims:
            ps = psp.tile([H, 1024], F32, name=f"ps{li}_{d}", tag="cps")
            nc.tensor.matmul(ps[:, 0:npts],
                             w_ap[:, w_off + d * wid:w_off + (d + 1) * wid],
                             h_cur[d], start=True, stop=True)
            h = work.tile([H, npts], F32R, name=f"h{li}_{d}", tag=f"h_{d}")
            nc.scalar.activation(h[:], ps[:, 0:npts], TANH,
                                 bias=wp_sb[:, b_off + d:b_off + d + 1])
            h_cur[d] = h
    # Last hidden layer + final layer fused per-dim so dim d's factor
    # tile is ready as early as possible (d0/d1 feed the KR stream; d2
    # is only needed by the first CP matmul). Final-layer bias-adds on
    # VectorE (idle during the head; ACT is busy with tanh).
    for d, _, npts in dims:
        ps = psp.tile([H, 1024], F32, name=f"ps2_{d}", tag="cps")
        nc.tensor.matmul(ps[:, 0:npts],
                         wp_r[:, W2_OFF + d * H:W2_OFF + (d + 1) * H],
                         h_cur[d], start=True, stop=True)
        h = work.tile([H, npts], F32R, name=f"h2_{d}", tag=f"h_{d}")
        nc.scalar.activation(h[:], ps[:, 0:npts], TANH,
                             bias=wp_sb[:, B2_OFF + d:B2_OFF + d + 1])
        w3d = wp_r[:, W3_OFF + d * H:W3_OFF + (d + 1) * H]
        psf = psp.tile([2 * R, 1024], F32, name=f"psf_{d}", tag="cps")
        nc.tensor.matmul(psf[:, 0:npts], w3d, h[:], start=True, stop=True)
        b3 = wp_sb[:, B3_OFF + d:B3_OFF + d + 1]
        if d == 0:
            half = NA // 2
            nc.vector.tensor_scalar_add(f0p[0:R, :], psf[0:R, 0:half],
                                        b3[0:R, :])
            nc.vector.tensor_scalar_add(f0p[R:2 * R, :],
                                        psf[R:2 * R, half:NA], b3[R:2 * R, :])
        else:
            f_sb = f1_sb if d == 1 else f2_sb
            nc.vector.tensor_scalar_add(f_sb[:], psf[:, 0:npts], b3)

    if variant == "mlp_only":
        sink = work.tile([2 * R, N], F32, name="sink", tag="sink")
        nc.vector.tensor_copy(sink[:], f2_sb[:])
        nc.vector.tensor_copy(sink[:], f1_sb[:])
        nc.vector.tensor_copy(sink[:, 0:NA // 2], f0p[:])
        return

    # Khatri-Rao: kr[r, a*N + b] = f0[r, a] * f1[r, b], f32r, both
    # output halves per op (low partitions: a = j, high: a = j + 20).
    # Emitted just-in-time per quad so the first copies aren't delayed.
    kr_sb = consts.tile([2 * R, NROWS // 2], F16)
    kr_emitted = 0
    kr_eng = nc.gpsimd if KR_ENGINE == "gpsimd" else nc.vector

    def emit_kr_upto(a_need):
        nonlocal kr_emitted
        while kr_emitted < min(a_need, NA // 2):
            j = kr_emitted
            kr_eng.tensor_scalar_mul(kr_sb[:, j * N:(j + 1) * N],
                                     f1_sb[:, :], f0p[:, j:j + 1])
            kr_emitted += 1

    if variant == "mlp_kr":
        emit_kr_upto(NA // 2)
        return

    # CP reconstruction: 25 quads in tapered DMA groups. Quad q covers
    # chunk pairs t0=2q, 2q+1 as ONE 4-bank PSUM tile [lo_t0 | lo_t1 |
    # hi_t0 | hi_t1], 4 matmuls, one 4-block strided copy into group
    # staging (lo region | hi region, each contiguous). Per group: lo
    # DMA on the SP ring; hi DMA on the ScalarE ring, but EMITTED one
    # quad into the next group so it never stalls ACT's in-order copy
    # queue while waiting for the group's last copy.
    pending_hi = None

    def flush_hi():
        nonlocal pending_hi
        if pending_hi is not None:
            nc.scalar.dma_start(*pending_hi)
            pending_hi = None

    q = 0
    for gsz in GROUPS_Q:
        stg = stagep.tile([MCH, 2 * GMAX * 2 * N], F16, name="stg",
                          tag="stg")
        for i in range(gsz):
            t = q + i
            emit_kr_upto(-(-((t + 2) * 2 * MCH) // N))
            kv = kr_sb[:, :].rearrange("r (x j) -> r j x", j=2)
            ps_lo = psp.tile([MCH, 1024], F32, name="cps_lo", tag="cps")
            ps_hi = psp.tile([MCH, 1024], F32, name="cps_hi", tag="cps")
            for k in (0, 1):
                nc.tensor.matmul(ps_lo[:, k * 512:k * 512 + N],
                                 kv[0:R, k, t * MCH:(t + 1) * MCH],
                                 f2_sb[0:R, :], start=True, stop=True)
                nc.tensor.matmul(ps_hi[:, k * 512:k * 512 + N],
                                 kv[R:2 * R, k, t * MCH:(t + 1) * MCH],
                                 f2_sb[R:2 * R, :], start=True, stop=True)
            if variant == "no_copy":
                continue
            src, dst = _pair_copy_views(ps_lo, stg, 0, i)
            (nc.vector.tensor_copy if COPY_ENG[2 * t] == 'v'
             else nc.scalar.copy)(dst, src)
            src, dst = _pair_copy_views(ps_hi, stg, 1, i)
            (nc.vector.tensor_copy if COPY_ENG[2 * t + 1] == 'v'
             else nc.scalar.copy)(dst, src)
            if i == min(2, gsz - 1) and variant not in ("no_dma",):
                flush_hi()
        if variant in ("no_copy", "no_dma"):
            q += gsz
            continue
        sv = stg[:, :].rearrange("p (s r) -> p s r", s=2)
        lo = sv[:, 0, 0:gsz * 2 * N].rearrange("p (m w) -> p m w", w=2 * N)
        hi = sv[:, 1, 0:gsz * 2 * N].rearrange("p (m w) -> p m w", w=2 * N)
        nc.sync.dma_start(outv[:, q:q + gsz, :], lo)
        pending_hi = (outv[:, NQUAD + q:NQUAD + q + gsz, :], hi)
        q += gsz
    flush_hi()


def _get_program():
    global _PROG
    if _PROG is None:
        _PROG = _build_program()
    return _PROG


def _pack_weights(W0, b0, W1, b1, W2, b2, W3, b3):
    wp = np.zeros((H, WCOLS), np.float32)
    for d in range(DIMS):
        wp[:, W1_OFF + d * H:W1_OFF + (d + 1) * H] = W1[d]
        wp[:, W2_OFF + d * H:W2_OFF + (d + 1) * H] = W2[d]
        wp[:, W3_OFF + d * H:W3_OFF + d * H + R] = W3[d]
        wp[:, W3_OFF + d * H + R:W3_OFF + (d + 1) * H] = W3[d]
        wp[:, B0_OFF + d] = b0[d]
        wp[:, B1_OFF + d] = b1[d]
        wp[:, B2_OFF + d] = b2[d]
        wp[0:R, B3_OFF + d] = b3[d]
        wp[R:2 * R, B3_OFF + d] = b3[d]
        wp[0, W0_OFF + d * H:W0_OFF + (d + 1) * H] = W0[d, 0]
    return wp


def _make_in_maps(xs, W0, b0, W1, b1, W2, b2, W3, b3):
    f = lambda x: np.ascontiguousarray(np.asarray(x), dtype=np.float32)
    xs = f(xs)
    wp = _pack_weights(f(W0), f(b0), f(W1), f(b1), f(W2), f(b2), f(W3), f(b3))
    in_maps = []
    for i in range(NCORES):
        w = wp.copy()
        w[0, XP_OFF + X0_OFF:XP_OFF + X0_OFF + NA] = xs[0, i * NA:(i + 1) * NA, 0]
        w[0, XP_OFF + X1_OFF:XP_OFF + X1_OFF + N] = xs[1, :, 0]
        w[0, XP_OFF + X2_OFF:XP_OFF + X2_OFF + N] = xs[2, :, 0]
        in_maps.append({"wp": w})
    return in_maps


def run_spmd(inputs_kwargs, **run_kwargs):
    """Build (cached) program, run on all 8 cores; returns BassKernelResults."""
    nc = _get_program()
    in_maps = _make_in_maps(**inputs_kwargs)
    return run_bass_kernel_spmd(nc, in_maps, core_ids=list(range(NCORES)),
                                **run_kwargs)


def kernel(xs, W0, b0, W1, b1, W2, b2, W3, b3):
    res = run_spmd(dict(xs=xs, W0=W0, b0=b0, W1=W1, b1=b1,
                        W2=W2, b2=b2, W3=W3, b3=b3))
    slabs = [r["out"].astype(np.float32).reshape(NA, N, N)
             for r in res.results]
    return np.concatenate(slabs, axis=0)


# revision 23
# speedup vs baseline: 1.1007x; 1.1007x over previous
"""Trainium2 Bass kernel for the CP-PINN tensor reconstruction problem.

Computes, for xs (3,320,1) and three per-axis MLP weight stacks:
    f_d = MLP_d(xs[d])            (320, 64)   [tanh MLP: 1->128->128->128->64]
    out[a,b,c] = sum_r f_0[a,r] * f_1[b,r] * f_2[c,r]   ->  (320, 320, 320) f32

Strategy: data-parallel over the output's first axis across 8 NeuronCores
(40 a-points per core, no collectives). The output stream is fp16 (the
measured rel-L2 error vs the f32 reference is ~6e-4, far under the 2e-2
gate), halving the HBM write floor from ~45.8us to ~22.9us per core.
Each core:
  - loads weights+biases+x with ONE host-packed DMA (x is packed into
    row 0 of the weight tensor), then makes a single f32r-rounded copy
    wp_r: fp32 matmuls run at 1/4 PE rate, float32r at full rate for
    free-dim >= 256, but every f32r-matmul input must be *produced*
    rounded (and f32r matmuls cannot write PSUM at a partition offset,
    so w3 is packed [w3 | w3] to fill both halves in one matmul);
  - computes the three MLPs (f32r matmuls on PE, tanh on ScalarE,
    interleaved layer-by-layer; last hidden + final layer fused per-dim),
    final-layer bias-adds on VectorE writing fp16 factor tiles
    duplicated across partition halves; f0 is packed as f0p (128, 20):
    rows 0-63 = f0[:, a], rows 64-127 = f0[:, a+20];
  - Khatri-Rao kr[r, a*N+b] = f0[r,a]*f1[r,b] in fp16 via 20 dual-half
    VectorE tensor_scalar_mul ops serving both output halves at once
    (GPSIMD was tried and is unusable: ~6us/op dispatch overhead);
  - reconstructs its (40*320, 320) fp16 slab in 25 "quads" = one
    256-row low chunk + one 256-row high chunk. Each partition owns TWO
    consecutive output rows (even/odd matmuls from stride-2 kr column
    views) so every DMA descriptor covers 1280 B of DRAM -- 640 B rows
    measured only ~180 GB/s vs ~330 GB/s at 1280 B. Per chunk: one
    2-bank PSUM pair-tile (4 in flight), 2 fp16 matmuls, one 2-block
    PSUM->fp16-SBUF copy (VectorE ~716ns / ScalarE ~1123ns, 29/21
    rate-balanced assignment; these copies are the only PSUM
    evacuation path and bound the stream together with the DMA),
    staged contiguously per stream; tapered DMA groups go out on BOTH
    HWDGE rings (one ring caps at ~220 GB/s): low rows from the
    otherwise-idle SP sequencer, high rows from ScalarE but emitted a
    few quads late so ACT's in-order queue never stalls on the wait.
"""

import sys

if "/opt/trn_rl_repo" not in sys.path:
    sys.path.insert(0, "/opt/trn_rl_repo")

import numpy as np

import concourse.bacc as bacc
import concourse.mybir as mybir
from concourse import tile
from concourse.bass_utils import run_bass_kernel_spmd

DIMS = 3
N = 320          # points per coordinate axis
R = 64           # CP rank
H = 128          # hidden width
NCORES = 8
NA = N // NCORES          # a-points per core (40)
NROWS = NA * N            # output rows per core (12800)
MCH = 128                 # (a,b)-rows per matmul chunk
NCHUNK = NROWS // MCH     # 100
NPAIR = NCHUNK // 2       # 50 low/high chunk pairs
NQUAD = NPAIR // 2        # 25 two-pair quads
GROUPS_Q = (1, 3, 6, 6, 5, 3, 1)   # quads per output DMA group
assert sum(GROUPS_Q) == NQUAD
GMAX = max(GROUPS_Q)

# Copy-engine assignment per PSUM pair-tile, sequence [lo_q0, hi_q0,
# lo_q1, hi_q1, ...]: 'v' = VectorE (~716ns/copy), 's' = ScalarE
# (~1123ns/copy). DVE also carries the KR stream + final adds; ACT the
# MLP head + hi-stream DMA issues. (A 4-bank/quad variant with single
# 4-block ACT copies measured WORSE: PSUM drops to 2 slots and the
# matmul->copy->matmul chain serializes.)
_NV = 29
COPY_ENG = tuple(
    'v' if i in {round(j * 50 / _NV) for j in range(_NV)} else 's'
    for i in range(50))

KR_ENGINE = "vector"   # "gpsimd" | "vector" (gpsimd: ~6us/op Q7 dispatch - unusable)

# Packed-weights column layout (one (128, WCOLS) f32 tensor):
#   [0,384)    w1 (3 x 128 cols)        [384,768)  w2
#   [768,1152) w3 duplicated: per dim 128 cols = [w3 | w3] so ONE f32r
#              matmul writes the factor into both partition halves
#              (f32r matmuls cannot target a PSUM partition offset)
#   [1152,1155) b0 [1155,1158) b1 [1158,1161) b2 [1161,1164) b3 (dup halves)
#   [1164,1548) w0 (row 0 only, 3 x 128 cols)   [1548,2228) packed x (row 0)
W1_OFF, W2_OFF, W3_OFF = 0, 384, 768
B0_OFF, B1_OFF, B2_OFF, B3_OFF = 1152, 1155, 1158, 1161
W0_OFF, WCOLS = 1164, 2228
XP_OFF = 1548
# Packed-x layout (row 0 of wp, from XP_OFF): x0(40) | x1(320) | x2(320)
X0_OFF, X1_OFF, X2_OFF, XCOLS = 0, NA, NA + N, NA + 2 * N

F32 = mybir.dt.float32
F32R = mybir.dt.float32r
F16 = mybir.dt.float16
TANH = mybir.ActivationFunctionType.Tanh

_PROG = None


def _build_program(loop=1, variant="full"):
    """loop>1 wraps the whole compute body in a Tile hardware For_i that
    repeats it `loop` times inside one NEFF launch — benchmarking only."""
    nc = bacc.Bacc("TRN2", target_bir_lowering=False)

    wp = nc.dram_tensor("wp", [H, WCOLS], F32, kind="ExternalInput")
    out = nc.dram_tensor("out", [NROWS, N], F16, kind="ExternalOutput")

    with tile.TileContext(nc) as tc:
        with (
            tc.tile_pool(name="consts", bufs=1) as consts,
            tc.tile_pool(name="work", bufs=2) as work,
            tc.tile_pool(name="stage", bufs=3) as stagep,
            tc.tile_pool(name="ps", bufs=4, space="PSUM") as psp,
        ):
            wp_sb = consts.tile([H, WCOLS], F32)
            nc.sync.dma_start(wp_sb[:], wp[:, :])
            # f32r-rounded copy: everything a matmul consumes (weights and
            # the packed x row) must be *produced* as f32r.
            wp_r = consts.tile([H, WCOLS], F32R)
            nc.vector.tensor_copy(wp_r[:], wp_sb[:])

            import contextlib
            loop_cm = (tc.For_i(0, loop, 1,
                                hint_engines=(mybir.EngineType.PE,),
                                staggered_reset=True)
                       if loop > 1 else contextlib.nullcontext())
            with loop_cm:
                _emit_body(nc, tc, consts, work, stagep, psp,
                           out, wp_sb, wp_r, variant)

    nc.compile()
    return nc


def _pair_copy_views(ps, stg, s, i):
    """(src, dst) for evacuating pair-tile ps (2 banks [even | odd], 320
    cols each at offsets 0/512) into stream region s (0=lo, 1=hi) slot i
    of the group staging tile — each DMA stream reads contiguously."""
    src = ps[:, :].rearrange("p (b x) -> p b x", x=512)[:, :, 0:N]
    dst = (stg[:, :].rearrange("p (s r) -> p s r", s=2)
           [:, s, i * 2 * N:(i + 1) * 2 * N]
           .rearrange("p (b c) -> p b c", c=N))
    return src, dst


def _emit_body(nc, tc, consts, work, stagep, psp, out, wp_sb, wp_r,
               variant="full"):
    # Each partition owns TWO consecutive output rows (j=0,1) so every
    # DMA descriptor covers 1280B of DRAM (640B rows halved throughput).
    outv = out[:, :].rearrange("(m p j) c -> p m (j c)", p=MCH, j=2)

    warm = work.tile([1, 1], F32, name="warm", tag="warm")
    nc.vector.memset(warm[:], 0.0)
    nc.scalar.activation(warm[:], warm[:], TANH)

    if variant == "empty":
        return

    if variant in ("dma_only", "dma_2ring", "cp_dve", "cp_act"):
        if variant in ("cp_dve", "cp_act"):
            ps0 = psp.tile([MCH, 1024], F32, name="ps0", tag="cps")
            for j in range(2):
                nc.scalar.copy(ps0[:, j * 512:(j + 1) * 512], wp_sb[:, 0:512])
        q = 0
        for gsz in GROUPS_Q:
            stg = stagep.tile([MCH, 2 * GMAX * 2 * N], F16, name="stg",
                              tag="stg")
            if variant in ("dma_only", "dma_2ring"):
                nc.vector.memset(stg[:, 0:1], 1.0)
            else:
                eng = (nc.vector.tensor_copy if variant == "cp_dve"
                       else nc.scalar.copy)
                for i in range(gsz):
                    for s in (0, 1):
                        src, dst = _pair_copy_views(ps0, stg, s, i)
                        eng(dst, src)
                q += gsz
                continue
            sv = stg[:, :].rearrange("p (s r) -> p s r", s=2)
            lo = sv[:, 0, 0:gsz * 2 * N].rearrange("p (m w) -> p m w", w=2 * N)
            hi = sv[:, 1, 0:gsz * 2 * N].rearrange("p (m w) -> p m w", w=2 * N)
            nc.sync.dma_start(outv[:, q:q + gsz, :], lo)
            (nc.scalar if variant == "dma_2ring" else nc.sync).dma_start(
                outv[:, NQUAD + q:NQUAD + q + gsz, :], hi)
            q += gsz
        return

    # f32 factor tiles, duplicated across both partition halves.
    # f0p: rows 0-63 = f0[:, j], rows 64-127 = f0[:, j+20].
    f0p = consts.tile([2 * R, NA // 2], F32)
    f1_sb = consts.tile([2 * R, N], F16)
    f2_sb = consts.tile([2 * R, N], F16)

    # The three MLPs interleaved layer-by-layer so PE never waits on the
    # ScalarEngine tanh of the same dim (PE executes in program order).
    dims = [(0, X0_OFF, NA), (1, X1_OFF, N), (2, X2_OFF, N)]
    h_cur = {d: wp_r[0:1, XP_OFF + xoff:XP_OFF + xoff + npts]
             for d, xoff, npts in dims}
    w_l0 = wp_r[0:1, :]
    for li, (w_off, b_off, w_ap, wid) in enumerate((
            (W0_OFF, B0_OFF, w_l0, H), (W1_OFF, B1_OFF, wp_r, H))):
        for d, _, npts in dims:
            ps = psp.tile([H, 1024], F32, name=f"ps{li}_{d}", tag="cps")
            nc.tensor.matmul(ps[:, 0:npts],
                             w_ap[:, w_off + d * wid:w_off + (d + 1) * wid],
                             h_cur[d], start=True, stop=True)
            h = work.tile([H, npts], F32R, name=f"h{li}_{d}", tag=f"h_{d}")
            nc.scalar.activation(h[:], ps[:, 0:npts], TANH,
                                 bias=wp_sb[:, b_off + d:b_off + d + 1])
            h_cur[d] = h
    # Last hidden layer + final layer fused per-dim so dim d's factor
    # tile is ready as early as possible (d0/d1 feed the KR stream; d2
    # is only needed by the first CP matmul). Final-layer bias-adds on
    # VectorE (idle during the head; ACT is busy with tanh).
    for d, _, npts in dims:
        ps = psp.tile([H, 1024], F32, name=f"ps2_{d}", tag="cps")
        nc.tensor.matmul(ps[:, 0:npts],
                         wp_r[:, W2_OFF + d * H:W2_OFF + (d + 1) * H],
                         h_cur[d], start=True, stop=True)
        h = work.tile([H, npts], F32R, name=f"h2_{d}", tag=f"h_{d}")
        nc.scalar.activation(h[:], ps[:, 0:npts], TANH,
                             bias=wp_sb[:, B2_OFF + d:B2_OFF + d + 1])
        w3d = wp_r[:, W3_OFF + d * H:W3_OFF + (d + 1) * H]
        psf = psp.tile([2 * R, 1024], F32, name=f"psf_{d}", tag="cps")
        nc.tensor.matmul(psf[:, 0:npts], w3d, h[:], start=True, stop=True)
        b3 = wp_sb[:, B3_OFF + d:B3_OFF + d + 1]
        if d == 0:
            half = NA // 2
            nc.vector.tensor_scalar_add(f0p[0:R, :], psf[0:R, 0:half],
                                        b3[0:R, :])
            nc.vector.tensor_scalar_add(f0p[R:2 * R, :],
                                        psf[R:2 * R, half:NA], b3[R:2 * R, :])
        else:
            f_sb = f1_sb if d == 1 else f2_sb
            nc.vector.tensor_scalar_add(f_sb[:], psf[:, 0:npts], b3)

    if variant == "mlp_only":
        sink = work.tile([2 * R, N], F32, name="sink", tag="sink")
        nc.vector.tensor_copy(sink[:], f2_sb[:])
        nc.vector.tensor_copy(sink[:], f1_sb[:])
        nc.vector.tensor_copy(sink[:, 0:NA // 2], f0p[:])
        return

    # Khatri-Rao: kr[r, a*N + b] = f0[r, a] * f1[r, b], f32r, both
    # output halves per op (low partitions: a = j, high: a = j + 20).
    # Emitted just-in-time per quad so the first copies aren't delayed.
    kr_sb = consts.tile([2 * R, NROWS // 2], F16)
    kr_emitted = 0
    kr_eng = nc.gpsimd if KR_ENGINE == "gpsimd" else nc.vector

    def emit_kr_upto(a_need):
        nonlocal kr_emitted
        while kr_emitted < min(a_need, NA // 2):
            j = kr_emitted
            kr_eng.tensor_scalar_mul(kr_sb[:, j * N:(j + 1) * N],
                                     f1_sb[:, :], f0p[:, j:j + 1])
            kr_emitted += 1

    if variant == "mlp_kr":
        emit_kr_upto(NA // 2)
        return

    # CP reconstruction: 25 quads in tapered DMA groups. Per quad: two
    # 2-bank PSUM pair-tiles (lo/hi; 4 in flight), 2 matmuls per tile
    # (even/odd output rows from stride-2 kr views), one 2-block copy
    # per tile into group staging (lo region | hi region, contiguous).
    # Per group: lo DMA on the SP HWDGE ring; hi DMA on the ScalarE
    # ring, but EMITTED one quad into the next group so it never stalls
    # ACT's in-order copy queue while waiting for the group's last copy.
    # (SWDGE hi-DMAs from the idle GPSIMD engine measured ~2us slower:
    # the Q7 issue latency delays the hi stream.)
    pending_hi = None

    def flush_hi():
        nonlocal pending_hi
        if pending_hi is not None:
            nc.scalar.dma_start(*pending_hi)
            pending_hi = None

    q = 0
    for gsz in GROUPS_Q:
        stg = stagep.tile([MCH, 2 * GMAX * 2 * N], F16, name="stg",
                          tag="stg")
        for i in range(gsz):
            t = q + i
            emit_kr_upto(-(-((t + 2) * 2 * MCH) // N))
            kv = kr_sb[:, :].rearrange("r (x j) -> r j x", j=2)
            ps_lo = psp.tile([MCH, 1024], F32, name="cps_lo", tag="cps")
            ps_hi = psp.tile([MCH, 1024], F32, name="cps_hi", tag="cps")
            for k in (0, 1):
                nc.tensor.matmul(ps_lo[:, k * 512:k * 512 + N],
                                 kv[0:R, k, t * MCH:(t + 1) * MCH],
                                 f2_sb[0:R, :], start=True, stop=True)
                nc.tensor.matmul(ps_hi[:, k * 512:k * 512 + N],
                                 kv[R:2 * R, k, t * MCH:(t + 1) * MCH],
                                 f2_sb[R:2 * R, :], start=True, stop=True)
            if variant == "no_copy":
                continue
            src, dst = _pair_copy_views(ps_lo, stg, 0, i)
            (nc.vector.tensor_copy if COPY_ENG[2 * t] == 'v'
             else nc.scalar.copy)(dst, src)
            src, dst = _pair_copy_views(ps_hi, stg, 1, i)
            (nc.vector.tensor_copy if COPY_ENG[2 * t + 1] == 'v'
             else nc.scalar.copy)(dst, src)
            if i == min(2, gsz - 1) and variant not in ("no_dma",):
                flush_hi()
        if variant in ("no_copy", "no_dma"):
            q += gsz
            continue
        sv = stg[:, :].rearrange("p (s r) -> p s r", s=2)
        lo = sv[:, 0, 0:gsz * 2 * N].rearrange("p (m w) -> p m w", w=2 * N)
        hi = sv[:, 1, 0:gsz * 2 * N].rearrange("p (m w) -> p m w", w=2 * N)
        nc.sync.dma_start(outv[:, q:q + gsz, :], lo)
        pending_hi = (outv[:, NQUAD + q:NQUAD + q + gsz, :], hi)
        q += gsz
    flush_hi()


def _get_program():
    global _PROG
    if _PROG is None:
        _PROG = _build_program()
    return _PROG


def _pack_weights(W0, b0, W1, b1, W2, b2, W3, b3):
    wp = np.zeros((H, WCOLS), np.float32)
    for d in range(DIMS):
        wp[:, W1_OFF + d * H:W1_OFF + (d + 1) * H] = W1[d]
        wp[:, W2_OFF + d * H:W2_OFF + (d + 1) * H] = W2[d]
        wp[:, W3_OFF + d * H:W3_OFF + d * H + R] = W3[d]
        wp[:, W3_OFF + d * H + R:W3_OFF + (d + 1) * H] = W3[d]
        wp[:, B0_OFF + d] = b0[d]
        wp[:, B1_OFF + d] = b1[d]
        wp[:, B2_OFF + d] = b2[d]
        wp[0:R, B3_OFF + d] = b3[d]
        wp[R:2 * R, B3_OFF + d] = b3[d]
        wp[0, W0_OFF + d * H:W0_OFF + (d + 1) * H] = W0[d, 0]
    return wp


def _make_in_maps(xs, W0, b0, W1, b1, W2, b2, W3, b3):
    f = lambda x: np.ascontiguousarray(np.asarray(x), dtype=np.float32)
    xs = f(xs)
    wp = _pack_weights(f(W0), f(b0), f(W1), f(b1), f(W2), f(b2), f(W3), f(b3))
    in_maps = []
    for i in range(NCORES):
        w = wp.copy()
        w[0, XP_OFF + X0_OFF:XP_OFF + X0_OFF + NA] = xs[0, i * NA:(i + 1) * NA, 0]
        w[0, XP_OFF + X1_OFF:XP_OFF + X1_OFF + N] = xs[1, :, 0]
        w[0, XP_OFF + X2_OFF:XP_OFF + X2_OFF + N] = xs[2, :, 0]
        in_maps.append({"wp": w})
    return in_maps


def run_spmd(inputs_kwargs, **run_kwargs):
    """Build (cached) program, run on all 8 cores; returns BassKernelResults."""
    nc = _get_program()
    in_maps = _make_in_maps(**inputs_kwargs)
    return run_bass_kernel_spmd(nc, in_maps, core_ids=list(range(NCORES)),
                                **run_kwargs)


def kernel(xs, W0, b0, W1, b1, W2, b2, W3, b3):
    res = run_spmd(dict(xs=xs, W0=W0, b0=b0, W1=W1, b1=b1,
                        W2=W2, b2=b2, W3=W3, b3=b3))
    slabs = [r["out"].astype(np.float32).reshape(NA, N, N)
             for r in res.results]
    return np.concatenate(slabs, axis=0)


# revision 24
# speedup vs baseline: 1.1069x; 1.0056x over previous
"""Trainium2 Bass kernel for the CP-PINN tensor reconstruction problem.

Computes, for xs (3,320,1) and three per-axis MLP weight stacks:
    f_d = MLP_d(xs[d])            (320, 64)   [tanh MLP: 1->128->128->128->64]
    out[a,b,c] = sum_r f_0[a,r] * f_1[b,r] * f_2[c,r]   ->  (320, 320, 320) f32

Strategy: data-parallel over the output's first axis across 8 NeuronCores
(40 a-points per core, no collectives). The output stream is fp16 (the
measured rel-L2 error vs the f32 reference is ~6e-4, far under the 2e-2
gate), halving the HBM write floor from ~45.8us to ~22.9us per core.
Each core:
  - loads weights+biases+x with ONE host-packed DMA (x is packed into
    row 0 of the weight tensor), then makes a single f32r-rounded copy
    wp_r: fp32 matmuls run at 1/4 PE rate, float32r at full rate for
    free-dim >= 256, but every f32r-matmul input must be *produced*
    rounded (and f32r matmuls cannot write PSUM at a partition offset,
    so w3 is packed [w3 | w3] to fill both halves in one matmul);
  - computes the three MLPs (f32r matmuls on PE, tanh on ScalarE,
    interleaved layer-by-layer; last hidden + final layer fused per-dim),
    final-layer bias-adds on VectorE writing fp16 factor tiles
    duplicated across partition halves; f0 is packed as f0p (128, 20):
    rows 0-63 = f0[:, a], rows 64-127 = f0[:, a+20];
  - Khatri-Rao kr[r, a*N+b] = f0[r,a]*f1[r,b] in fp16 via 20 dual-half
    VectorE tensor_scalar_mul ops serving both output halves at once
    (GPSIMD was tried and is unusable: ~6us/op dispatch overhead);
  - reconstructs its (40*320, 320) fp16 slab in 25 "quads" = one
    256-row low chunk + one 256-row high chunk. Each partition owns TWO
    consecutive output rows (even/odd matmuls from stride-2 kr column
    views) so every DMA descriptor covers 1280 B of DRAM -- 640 B rows
    measured only ~180 GB/s vs ~330 GB/s at 1280 B. Per chunk: one
    2-bank PSUM pair-tile (4 in flight), 2 fp16 matmuls, one 2-block
    PSUM->fp16-SBUF copy (VectorE ~716ns / ScalarE ~1123ns, 29/21
    rate-balanced assignment; these copies are the only PSUM
    evacuation path and bound the stream together with the DMA),
    staged contiguously per stream; tapered DMA groups go out on BOTH
    HWDGE rings (one ring caps at ~220 GB/s): low rows from the
    otherwise-idle SP sequencer, high rows from ScalarE but emitted a
    few quads late so ACT's in-order queue never stalls on the wait.
"""

import sys

if "/opt/trn_rl_repo" not in sys.path:
    sys.path.insert(0, "/opt/trn_rl_repo")

import numpy as np

import concourse.bacc as bacc
import concourse.mybir as mybir
from concourse import tile
from concourse.bass_utils import run_bass_kernel_spmd

DIMS = 3
N = 320          # points per coordinate axis
R = 64           # CP rank
H = 128          # hidden width
NCORES = 8
NA = N // NCORES          # a-points per core (40)
NROWS = NA * N            # output rows per core (12800)
MCH = 128                 # (a,b)-rows per matmul chunk
NCHUNK = NROWS // MCH     # 100
NPAIR = NCHUNK // 2       # 50 low/high chunk pairs
NQUAD = NPAIR // 2        # 25 two-pair quads
GROUPS_Q = (1, 3, 6, 6, 5, 3, 1)   # quads per output DMA group
assert sum(GROUPS_Q) == NQUAD
GMAX = max(GROUPS_Q)

# Copy-engine assignment per PSUM pair-tile, sequence [lo_q0, hi_q0,
# lo_q1, hi_q1, ...]: 'v' = VectorE (~716ns/copy), 's' = ScalarE
# (~1123ns/copy). DVE also carries the KR stream + final adds; ACT the
# MLP head + hi-stream DMA issues. (A 4-bank/quad variant with single
# 4-block ACT copies measured WORSE: PSUM drops to 2 slots and the
# matmul->copy->matmul chain serializes.)
_NV = 29
COPY_ENG = tuple(
    'v' if i in {round(j * 50 / _NV) for j in range(_NV)} else 's'
    for i in range(50))

KR_ENGINE = "vector"   # "gpsimd" | "vector" (gpsimd: ~6us/op Q7 dispatch - unusable)

# Packed-weights column layout (one (128, WCOLS) f32 tensor):
#   [0,384)    w1 (3 x 128 cols)        [384,768)  w2
#   [768,1152) w3 duplicated: per dim 128 cols = [w3 | w3] so ONE f32r
#              matmul writes the factor into both partition halves
#              (f32r matmuls cannot target a PSUM partition offset)
#   [1152,1155) b0 [1155,1158) b1 [1158,1161) b2 [1161,1164) b3 (dup halves)
#   [1164,1548) w0 (row 0 only, 3 x 128 cols)   [1548,2228) packed x (row 0)
W1_OFF, W2_OFF, W3_OFF = 0, 384, 768
B0_OFF, B1_OFF, B2_OFF, B3_OFF = 1152, 1155, 1158, 1161
W0_OFF, WCOLS = 1164, 2228
XP_OFF = 1548
# Packed-x layout (row 0 of wp, from XP_OFF): x0(40) | x1(320) | x2(320)
X0_OFF, X1_OFF, X2_OFF, XCOLS = 0, NA, NA + N, NA + 2 * N

F32 = mybir.dt.float32
F32R = mybir.dt.float32r
F16 = mybir.dt.float16
TANH = mybir.ActivationFunctionType.Tanh

_PROG = None


def _build_program(loop=1, variant="full"):
    """loop>1 wraps the whole compute body in a Tile hardware For_i that
    repeats it `loop` times inside one NEFF launch — benchmarking only."""
    nc = bacc.Bacc("TRN2", target_bir_lowering=False)

    wp = nc.dram_tensor("wp", [H, WCOLS], F32, kind="ExternalInput")
    out = nc.dram_tensor("out", [NROWS, N], F16, kind="ExternalOutput")

    with tile.TileContext(nc) as tc:
        with (
            tc.tile_pool(name="consts", bufs=1) as consts,
            tc.tile_pool(name="work", bufs=2) as work,
            tc.tile_pool(name="stage", bufs=4) as stagep,
            tc.tile_pool(name="ps", bufs=4, space="PSUM") as psp,
        ):
            wp_sb = consts.tile([H, WCOLS], F32)
            nc.sync.dma_start(wp_sb[:], wp[:, :])
            # f32r-rounded copy: everything a matmul consumes (weights and
            # the packed x row) must be *produced* as f32r.
            wp_r = consts.tile([H, WCOLS], F32R)
            nc.vector.tensor_copy(wp_r[:], wp_sb[:])

            import contextlib
            loop_cm = (tc.For_i(0, loop, 1,
                                hint_engines=(mybir.EngineType.PE,),
                                staggered_reset=True)
                       if loop > 1 else contextlib.nullcontext())
            with loop_cm:
                _emit_body(nc, tc, consts, work, stagep, psp,
                           out, wp_sb, wp_r, variant)

    nc.compile()
    return nc


def _pair_copy_views(ps, stg, s, i):
    """(src, dst) for evacuating pair-tile ps (2 banks [even | odd], 320
    cols each at offsets 0/512) into stream region s (0=lo, 1=hi) slot i
    of the group staging tile — each DMA stream reads contiguously."""
    src = ps[:, :].rearrange("p (b x) -> p b x", x=512)[:, :, 0:N]
    dst = (stg[:, :].rearrange("p (s r) -> p s r", s=2)
           [:, s, i * 2 * N:(i + 1) * 2 * N]
           .rearrange("p (b c) -> p b c", c=N))
    return src, dst


def _emit_body(nc, tc, consts, work, stagep, psp, out, wp_sb, wp_r,
               variant="full"):
    # Each partition owns TWO consecutive output rows (j=0,1) so every
    # DMA descriptor covers 1280B of DRAM (640B rows halved throughput).
    outv = out[:, :].rearrange("(m p j) c -> p m (j c)", p=MCH, j=2)

    warm = work.tile([1, 1], F32, name="warm", tag="warm")
    nc.vector.memset(warm[:], 0.0)
    nc.scalar.activation(warm[:], warm[:], TANH)

    if variant == "empty":
        return

    if variant in ("dma_only", "dma_2ring", "cp_dve", "cp_act"):
        if variant in ("cp_dve", "cp_act"):
            ps0 = psp.tile([MCH, 1024], F32, name="ps0", tag="cps")
            for j in range(2):
                nc.scalar.copy(ps0[:, j * 512:(j + 1) * 512], wp_sb[:, 0:512])
        q = 0
        for gsz in GROUPS_Q:
            stg = stagep.tile([MCH, 2 * GMAX * 2 * N], F16, name="stg",
                              tag="stg")
            if variant in ("dma_only", "dma_2ring"):
                nc.vector.memset(stg[:, 0:1], 1.0)
            else:
                eng = (nc.vector.tensor_copy if variant == "cp_dve"
                       else nc.scalar.copy)
                for i in range(gsz):
                    for s in (0, 1):
                        src, dst = _pair_copy_views(ps0, stg, s, i)
                        eng(dst, src)
                q += gsz
                continue
            sv = stg[:, :].rearrange("p (s r) -> p s r", s=2)
            lo = sv[:, 0, 0:gsz * 2 * N].rearrange("p (m w) -> p m w", w=2 * N)
            hi = sv[:, 1, 0:gsz * 2 * N].rearrange("p (m w) -> p m w", w=2 * N)
            nc.sync.dma_start(outv[:, q:q + gsz, :], lo)
            (nc.scalar if variant == "dma_2ring" else nc.sync).dma_start(
                outv[:, NQUAD + q:NQUAD + q + gsz, :], hi)
            q += gsz
        return

    # f32 factor tiles, duplicated across both partition halves.
    # f0p: rows 0-63 = f0[:, j], rows 64-127 = f0[:, j+20].
    f0p = consts.tile([2 * R, NA // 2], F32)
    f1_sb = consts.tile([2 * R, N], F16)
    f2_sb = consts.tile([2 * R, N], F16)

    # The three MLPs interleaved layer-by-layer so PE never waits on the
    # ScalarEngine tanh of the same dim (PE executes in program order).
    dims = [(0, X0_OFF, NA), (1, X1_OFF, N), (2, X2_OFF, N)]
    h_cur = {d: wp_r[0:1, XP_OFF + xoff:XP_OFF + xoff + npts]
             for d, xoff, npts in dims}
    w_l0 = wp_r[0:1, :]
    for li, (w_off, b_off, w_ap, wid) in enumerate((
            (W0_OFF, B0_OFF, w_l0, H), (W1_OFF, B1_OFF, wp_r, H))):
        for d, _, npts in dims:
            ps = psp.tile([H, 1024], F32, name=f"ps{li}_{d}", tag="cps")
            nc.tensor.matmul(ps[:, 0:npts],
                             w_ap[:, w_off + d * wid:w_off + (d + 1) * wid],
                             h_cur[d], start=True, stop=True)
            h = work.tile([H, npts], F32R, name=f"h{li}_{d}", tag=f"h_{d}")
            nc.scalar.activation(h[:], ps[:, 0:npts], TANH,
                                 bias=wp_sb[:, b_off + d:b_off + d + 1])
            h_cur[d] = h
    # Last hidden layer + final layer fused per-dim so dim d's factor
    # tile is ready as early as possible (d0/d1 feed the KR stream; d2
    # is only needed by the first CP matmul). Final-layer bias-adds on
    # VectorE (idle during the head; ACT is busy with tanh).
    for d, _, npts in dims:
        ps = psp.tile([H, 1024], F32, name=f"ps2_{d}", tag="cps")
        nc.tensor.matmul(ps[:, 0:npts],
                         wp_r[:, W2_OFF + d * H:W2_OFF + (d + 1) * H],
                         h_cur[d], start=True, stop=True)
        h = work.tile([H, npts], F32R, name=f"h2_{d}", tag=f"h_{d}")
        nc.scalar.activation(h[:], ps[:, 0:npts], TANH,
                             bias=wp_sb[:, B2_OFF + d:B2_OFF + d + 1])
        w3d = wp_r[:, W3_OFF + d * H:W3_OFF + (d + 1) * H]
        psf = psp.tile([2 * R, 1024], F32, name=f"psf_{d}", tag="cps")
        nc.tensor.matmul(psf[:, 0:npts], w3d, h[:], start=True, stop=True)
        b3 = wp_sb[:, B3_OFF + d:B3_OFF + d + 1]
        if d == 0:
            half = NA // 2
            nc.vector.tensor_scalar_add(f0p[0:R, :], psf[0:R, 0:half],
                                        b3[0:R, :])
            nc.vector.tensor_scalar_add(f0p[R:2 * R, :],
                                        psf[R:2 * R, half:NA], b3[R:2 * R, :])
        else:
            f_sb = f1_sb if d == 1 else f2_sb
            nc.vector.tensor_scalar_add(f_sb[:], psf[:, 0:npts], b3)

    if variant == "mlp_only":
        sink = work.tile([2 * R, N], F32, name="sink", tag="sink")
        nc.vector.tensor_copy(sink[:], f2_sb[:])
        nc.vector.tensor_copy(sink[:], f1_sb[:])
        nc.vector.tensor_copy(sink[:, 0:NA // 2], f0p[:])
        return

    # Khatri-Rao: kr[r, a*N + b] = f0[r, a] * f1[r, b], f32r, both
    # output halves per op (low partitions: a = j, high: a = j + 20).
    # Emitted just-in-time per quad so the first copies aren't delayed.
    kr_sb = consts.tile([2 * R, NROWS // 2], F16)
    kr_emitted = 0
    kr_eng = nc.gpsimd if KR_ENGINE == "gpsimd" else nc.vector

    def emit_kr_upto(a_need):
        nonlocal kr_emitted
        while kr_emitted < min(a_need, NA // 2):
            j = kr_emitted
            kr_eng.tensor_scalar_mul(kr_sb[:, j * N:(j + 1) * N],
                                     f1_sb[:, :], f0p[:, j:j + 1])
            kr_emitted += 1

    if variant == "mlp_kr":
        emit_kr_upto(NA // 2)
        return

    # CP reconstruction: 25 quads in tapered DMA groups. Per quad: two
    # 2-bank PSUM pair-tiles (lo/hi; 4 in flight), 2 matmuls per tile
    # (even/odd output rows from stride-2 kr views), one 2-block copy
    # per tile into group staging (lo region | hi region, contiguous).
    # Per group: lo DMA on the SP HWDGE ring; hi DMA on the ScalarE
    # ring, but EMITTED one quad into the next group so it never stalls
    # ACT's in-order copy queue while waiting for the group's last copy.
    # (SWDGE hi-DMAs from the idle GPSIMD engine measured ~2us slower:
    # the Q7 issue latency delays the hi stream.)
    pending_hi = None

    def flush_hi():
        nonlocal pending_hi
        if pending_hi is not None:
            nc.scalar.dma_start(*pending_hi)
            pending_hi = None

    q = 0
    for gsz in GROUPS_Q:
        stg = stagep.tile([MCH, 2 * GMAX * 2 * N], F16, name="stg",
                          tag="stg")
        for i in range(gsz):
            t = q + i
            emit_kr_upto(-(-((t + 2) * 2 * MCH) // N))
            kv = kr_sb[:, :].rearrange("r (x j) -> r j x", j=2)
            ps_lo = psp.tile([MCH, 1024], F32, name="cps_lo", tag="cps")
            ps_hi = psp.tile([MCH, 1024], F32, name="cps_hi", tag="cps")
            for k in (0, 1):
                nc.tensor.matmul(ps_lo[:, k * 512:k * 512 + N],
                                 kv[0:R, k, t * MCH:(t + 1) * MCH],
                                 f2_sb[0:R, :], start=True, stop=True)
                nc.tensor.matmul(ps_hi[:, k * 512:k * 512 + N],
                                 kv[R:2 * R, k, t * MCH:(t + 1) * MCH],
                                 f2_sb[R:2 * R, :], start=True, stop=True)
            if variant == "no_copy":
                continue
            src, dst = _pair_copy_views(ps_lo, stg, 0, i)
            (nc.vector.tensor_copy if COPY_ENG[2 * t] == 'v'
             else nc.scalar.copy)(dst, src)
            src, dst = _pair_copy_views(ps_hi, stg, 1, i)
            (nc.vector.tensor_copy if COPY_ENG[2 * t + 1] == 'v'
             else nc.scalar.copy)(dst, src)
            if i == min(2, gsz - 1) and variant not in ("no_dma",):
                flush_hi()
        if variant in ("no_copy", "no_dma"):
            q += gsz
            continue
        sv = stg[:, :].rearrange("p (s r) -> p s r", s=2)
        lo = sv[:, 0, 0:gsz * 2 * N].rearrange("p (m w) -> p m w", w=2 * N)
        hi = sv[:, 1, 0:gsz * 2 * N].rearrange("p (m w) -> p m w", w=2 * N)
        nc.sync.dma_start(outv[:, q:q + gsz, :], lo)
        pending_hi = (outv[:, NQUAD + q:NQUAD + q + gsz, :], hi)
        q += gsz
    flush_hi()


def _get_program():
    global _PROG
    if _PROG is None:
        _PROG = _build_program()
    return _PROG


def _pack_weights(W0, b0, W1, b1, W2, b2, W3, b3):
    wp = np.zeros((H, WCOLS), np.float32)
    for d in range(DIMS):
        wp[:, W1_OFF + d * H:W1_OFF + (d + 1) * H] = W1[d]
        wp[:, W2_OFF + d * H:W2_OFF + (d + 1) * H] = W2[d]
        wp[:, W3_OFF + d * H:W3_OFF + d * H + R] = W3[d]
        wp[:, W3_OFF + d * H + R:W3_OFF + (d + 1) * H] = W3[d]
        wp[:, B0_OFF + d] = b0[d]
        wp[:, B1_OFF + d] = b1[d]
        wp[:, B2_OFF + d] = b2[d]
        wp[0:R, B3_OFF + d] = b3[d]
        wp[R:2 * R, B3_OFF + d] = b3[d]
        wp[0, W0_OFF + d * H:W0_OFF + (d + 1) * H] = W0[d, 0]
    return wp


def _make_in_maps(xs, W0, b0, W1, b1, W2, b2, W3, b3):
    f = lambda x: np.ascontiguousarray(np.asarray(x), dtype=np.float32)
    xs = f(xs)
    wp = _pack_weights(f(W0), f(b0), f(W1), f(b1), f(W2), f(b2), f(W3), f(b3))
    in_maps = []
    for i in range(NCORES):
        w = wp.copy()
        w[0, XP_OFF + X0_OFF:XP_OFF + X0_OFF + NA] = xs[0, i * NA:(i + 1) * NA, 0]
        w[0, XP_OFF + X1_OFF:XP_OFF + X1_OFF + N] = xs[1, :, 0]
        w[0, XP_OFF + X2_OFF:XP_OFF + X2_OFF + N] = xs[2, :, 0]
        in_maps.append({"wp": w})
    return in_maps


def run_spmd(inputs_kwargs, **run_kwargs):
    """Build (cached) program, run on all 8 cores; returns BassKernelResults."""
    nc = _get_program()
    in_maps = _make_in_maps(**inputs_kwargs)
    return run_bass_kernel_spmd(nc, in_maps, core_ids=list(range(NCORES)),
                                **run_kwargs)


def kernel(xs, W0, b0, W1, b1, W2, b2, W3, b3):
    res = run_spmd(dict(xs=xs, W0=W0, b0=b0, W1=W1, b1=b1,
                        W2=W2, b2=b2, W3=W3, b3=b3))
    slabs = [r["out"].astype(np.float32).reshape(NA, N, N)
             for r in res.results]
    return np.concatenate(slabs, axis=0)
